# revision 19
# baseline (speedup 1.0000x reference)
"""Trainium2 Bass kernel for the ExemplarModel (Mahalanobis-kNN attention).

Reference math (N=1024 queries, M=50000 exemplars, D=512, C=10 classes):
    dist[n,m]  = sum_d Sigma_inv[d] * (x[n,d] - e[m,d])^2
    att[n,m]   = exp(-beta * dist[n,m])
    logits[n,c]= segment_sum(att over exemplars with label c)
    out        = softmax(gamma * logits, axis=1)

Distribution: exemplars/labels sharded along M across 8 NeuronCores
(6250 each, zero-padded to 6272 = 49*128); x, Sigma_inv, beta replicated.
Each core computes partial per-class logits
    P[c,n] = sum_m onehot[m,c] * exp(2*beta*cross[n,m] - beta*e_sq[m])
with cross[m,n] = sum_d e[m,d] * (x*Sigma_inv)[n,d].

v2 design (from trace analysis of the 89us v1):
  - e_sq is computed on the HOST (tiny: M*D mults) and shipped as per-tile
    bias columns; the raw f32 exemplar stream, its SWDGE cast-DMA (41us of
    DMA busy) and all DVE square/scale/reduce work (59us busy) are gone.
  - eT is retiled on the host into [128, t*512] so each tile is a
    contiguous 512B-per-partition run; groups of 4 tiles per DMA (2KB
    runs) replace v1's 242B-descriptor loads. First matmul can start at
    ~1us instead of ~13us.
  - exp is split across engines: even tiles on ScalarE (exact Exp -> fp8),
    odd tiles on DVE via a Schraudolph-style trick: bits =
    sat_u8(rne(16*beta*log2e*cross + 56 - 0.46 - 8*log2e*beta*e_sq))
    written as uint8 and bitcast to fp8e4 (HW convert saturates negatives
    to 0 == exp underflow). Each engine handles ~25 tiles (~30us) instead
    of ScalarE doing all 49 (~60us).
  - PE warmup matmuls run during the initial DMA fill to start the DVFS
    ramp (0.65 -> 1.2 -> 2.4 GHz) early.
The PE is the bottleneck: 196 cross + 50 segment DR fp8 matmuls at
~213ns steady (1 col/cycle, the real fp8 peak) ~= 52us.

The host combines: logits[n,c] = exp(-beta*x_sq[n]) * sum_cores P, then
gamma + softmax on the tiny [1024,10] result.
"""

import numpy as np
import ml_dtypes

import concourse.bass as bass
import concourse.bacc as bacc
import concourse.tile as tile
from concourse import mybir
from concourse import bass_utils

# Problem constants (hardcoded per contract; kernel.py must be self-contained).
N = 1024          # queries
M = 50000         # exemplars (global)
D = 512           # feature dim
C = 10            # classes
N_CORES = 8
M_LOC = M // N_CORES          # 6250 exemplars per core
P = 128                       # partitions
T_TILES = (M_LOC + P - 1) // P  # 49 tiles per core
M_PAD = T_TILES * P           # 6272
KC = D // P                   # 4 contraction chunks
CP = 16                       # one-hot pitch (fp8 DoubleRow needs step%16==0)
NH = N // 512                 # 2 matmul free-dim halves
G_TILES = 4                   # eT tiles per DMA group
N_GROUPS = (T_TILES + G_TILES - 1) // G_TILES
N_WARM = 20                   # PE warmup matmuls during DMA fill

LOG2E = float(np.log2(np.e))
DELTA = -0.46                 # Schraudolph magic offset for e4m3 (tuned)

FP32 = mybir.dt.float32
FP8 = mybir.dt.float8e4
U8 = mybir.dt.uint8
NP_FP8 = ml_dtypes.float8_e4m3


def build_nc(t_tiles=T_TILES, n=N, debug=False):
    """Build the per-core Bass program (SPMD: same program, per-core data)."""
    nc = bacc.Bacc("TRN2", target_bir_lowering=False, debug=debug,
                   num_devices=N_CORES)
    nh = n // 512

    eTt_dram = nc.dram_tensor("eTt", [P, t_tiles * D], FP8, kind="ExternalInput")
    xsT_dram = nc.dram_tensor("xsT", [D, n], FP8, kind="ExternalInput")
    w_dram = nc.dram_tensor("w", [P, t_tiles * CP], FP8, kind="ExternalInput")
    # cb = [ba | bd | sc] packed: one DMA covers every f32 constant
    cb_dram = nc.dram_tensor("cb", [P, 2 * t_tiles + 2], FP32,
                             kind="ExternalInput")
    out_dram = nc.dram_tensor("out", [C, n], FP32, kind="ExternalOutput")

    with tile.TileContext(nc) as tc:
        with (
            tc.tile_pool(name="const", bufs=1) as const_pool,
            tc.tile_pool(name="att", bufs=4) as att_pool,
            tc.tile_pool(name="crossp", bufs=3, space="PSUM") as cross_pool,
            tc.tile_pool(name="logitp", bufs=1, space="PSUM") as logit_pool,
        ):
            # ---- one-time preamble ----
            # xsT first on the Scalar HWDGE queue (feeds the first matmul);
            # one tile per 256-d pair chunk so the first matmul only waits
            # on chunks 0-1.
            # Scalar-queue order is latency-driven: the first cross matmul
            # needs xsT chunk-pair 0, then pair 1, then the act constants,
            # then w (first needed by the t==2 segment matmul). One DMA
            # each to minimize issue overhead and semaphore chaining.
            xsT_p0 = const_pool.tile([P, 2 * n], FP8, tag="xsTp0")
            xsT_p1 = const_pool.tile([P, 2 * n], FP8, tag="xsTp1")
            xsT_tiles = [xsT_p0, xsT_p1]
            xsT_view = xsT_dram.ap().rearrange("(k p) n -> p k n", p=P)
            ba = const_pool.tile([P, t_tiles], FP32, tag="ba")
            bd = const_pool.tile([P, t_tiles], FP32, tag="bd")
            sc = const_pool.tile([P, 2], FP32, tag="sc")
            w_f8 = const_pool.tile([P, t_tiles * CP], FP8, tag="w8")
            nc.scalar.dma_start(sc[:], cb_dram[:, 2 * t_tiles:2 * t_tiles + 2])
            nc.scalar.dma_start(
                xsT_p0[:].rearrange("p (k n) -> p k n", n=n),
                xsT_view[:, 0:2, :])
            nc.scalar.dma_start(ba[:], cb_dram[:, 0:t_tiles])
            nc.scalar.dma_start(bd[:], cb_dram[:, t_tiles:2 * t_tiles])
            nc.scalar.dma_start(
                xsT_p1[:].rearrange("p (k n) -> p k n", n=n),
                xsT_view[:, 2:4, :])
            nc.scalar.dma_start(w_f8[:], w_dram[:])
            xsT_pair_aps = [t_[:].rearrange("p (k n) -> p k n", n=n)
                            for t_ in xsT_tiles]

            # Tiled exemplar loads on the Sync HWDGE queue: graded group
            # sizes — small first groups so the early tiles land with low
            # latency, big groups later for issue/semaphore efficiency.
            group_sizes = [1, 2, 4, 8]
            while sum(group_sizes) + 8 <= t_tiles:
                group_sizes.append(8)
            rem = t_tiles - sum(group_sizes)
            if rem:
                group_sizes.append(rem)
            eT_groups = []
            tile2group = []
            off = 0
            for g, gt in enumerate(group_sizes):
                tile_g = const_pool.tile([P, gt * D], FP8, tag=f"eT{g}")
                nc.sync.dma_start(
                    tile_g[:], eTt_dram[:, off * D:(off + gt) * D])
                for lo in range(gt):
                    tile2group.append((g, lo))
                eT_groups.append(tile_g)
                off += gt

            # Full-width PSUM tile: [:CP] is the logits accumulator; the
            # warmup matmuls scribble on it first (the t==2 start=True
            # segment matmul resets its region afterwards).
            logits_full = logit_pool.tile([P, n], FP32)
            logits_ps = logits_full[:CP, :]

            # PE warmup: narrow DR matmuls on a zeroed scratch tile to start
            # the clock ramp while the first DMAs land. Sized to bridge the
            # PE from the post-preamble start (~6.9us) to first-data-ready
            # (~9.5us) without a gap (the DVFS clock decays within ~1us of
            # idle), while cheap enough not to delay the first real matmul.
            scratch = const_pool.tile([P, 2 * P], FP8, tag="scr")
            nc.gpsimd.memset(scratch[:], 0)
            scr_pairs = scratch[:].rearrange("p (i n) -> p i n", i=2)
            for _ in range(N_WARM):
                nc.tensor.matmul(
                    logits_full[:32, :P], lhsT=scr_pairs[:, :, :32],
                    rhs=scr_pairs, start=True, stop=True,
                    perf_mode=mybir.MatmulPerfMode.DoubleRow,
                    skip_group_check=True)

            w_pairs = w_f8[:].rearrange("p (t c) -> p t c", c=CP)

            # ---- main loop over exemplar tiles ----
            # att for two consecutive tiles shares one buffer so the segment
            # matmul can consume both via one fp8 DoubleRow op; it is issued
            # two tiles behind so the PE never waits on the act engines.
            att_pairs = []
            att_cur = None
            for t in range(t_tiles):
                g, lo = tile2group[t]
                eT_t = eT_groups[g][:, lo * D:(lo + 1) * D].rearrange(
                    "p (k m) -> p k m", m=P)

                # cross[m, n] = sum_d e[m,d] * xs[n,d]
                # fp8 DoubleRow: each matmul consumes a pair of 128-d chunks
                cross_ps = cross_pool.tile([P, n], FP32, tag="cross")
                for j in range(KC // 2):
                    for h in range(nh):
                        nc.tensor.matmul(
                            cross_ps[:, h * 512:(h + 1) * 512],
                            lhsT=eT_t[:, 2 * j:2 * j + 2, :],
                            rhs=xsT_pair_aps[j][:, :, h * 512:(h + 1) * 512],
                            start=(j == 0), stop=(j == KC // 2 - 1),
                            perf_mode=mybir.MatmulPerfMode.DoubleRow)

                # logits[c, n] += onehot[m, c]^T @ att[m, n]  (pair t//2 - 1)
                if t % 2 == 0 and len(att_pairs) >= 1 and not att_pairs[-1][1]:
                    p_idx, _ = att_pairs[-1]
                    att_pairs[-1] = (p_idx, True)
                    pr = p_idx[:].rearrange("p (i n) -> p i n", i=2)
                    for h in range(nh):
                        nc.tensor.matmul(
                            logits_ps[:, h * 512:(h + 1) * 512],
                            lhsT=w_pairs[:, t - 2:t, :],
                            rhs=pr[:, :, h * 512:(h + 1) * 512],
                            start=(t == 2), stop=False,
                            perf_mode=mybir.MatmulPerfMode.DoubleRow,
                            skip_group_check=True)

                # att = exp(2*beta*cross - beta*e_sq), alternating engines:
                # even tiles exact Exp on ScalarE (fp8 out), odd tiles
                # Schraudolph bits on DVE (uint8 out, bitcast fp8).
                if t % 2 == 0:
                    att_cur = att_pool.tile([P, 2 * n], FP8, tag="att")
                    att_pairs.append((att_cur, False))
                half = att_cur[:, (t % 2) * n:(t % 2 + 1) * n]
                if t % 2 == 0:
                    nc.scalar.activation(half, cross_ps[:],
                                         mybir.ActivationFunctionType.Exp,
                                         bias=ba[:, t:t + 1],
                                         scale=sc[:, 0:1])
                else:
                    nc.vector.tensor_scalar(
                        half.bitcast(U8), cross_ps[:],
                        sc[:, 1:2], bd[:, t:t + 1],
                        mybir.AluOpType.mult, mybir.AluOpType.add)

            # drain remaining segment matmuls
            n_pairs = t_tiles // 2
            last_single = (t_tiles % 2 == 1)
            for pi in range(len(att_pairs)):
                p_idx, done = att_pairs[pi]
                if done:
                    continue
                if pi < n_pairs:
                    pr = p_idx[:].rearrange("p (i n) -> p i n", i=2)
                    for h in range(nh):
                        nc.tensor.matmul(
                            logits_ps[:, h * 512:(h + 1) * 512],
                            lhsT=w_pairs[:, 2 * pi:2 * pi + 2, :],
                            rhs=pr[:, :, h * 512:(h + 1) * 512],
                            start=(pi == 0),
                            stop=(not last_single and pi == len(att_pairs) - 1),
                            perf_mode=mybir.MatmulPerfMode.DoubleRow,
                            skip_group_check=True)
                else:  # leftover single tile (first half of the pair buffer)
                    out_sb = const_pool.tile([C, n], FP32, tag="out")
                    for h in range(nh):
                        nc.tensor.matmul(
                            logits_ps[:, h * 512:(h + 1) * 512],
                            lhsT=w_f8[:, (t_tiles - 1) * CP:t_tiles * CP],
                            rhs=p_idx[:, h * 512:(h + 1) * 512],
                            start=False, stop=(h == nh - 1),
                            skip_group_check=True)
                        # epilogue for this n-half overlaps the next half's
                        # segment matmul
                        nc.vector.tensor_copy(
                            out_sb[:, h * 512:(h + 1) * 512],
                            logits_ps[:C, h * 512:(h + 1) * 512])
                        nc.sync.dma_start(
                            out_dram[:, h * 512:(h + 1) * 512],
                            out_sb[:, h * 512:(h + 1) * 512])

    nc.compile()
    return nc


def make_in_maps(x, exemplars, labels, Sigma_inv, beta, gamma,
                 t_tiles=T_TILES):
    """Shard the full inputs into per-core in_maps (host-side glue)."""
    x = np.asarray(x, dtype=np.float32)
    exemplars = np.asarray(exemplars, dtype=np.float32)
    labels = np.asarray(labels).astype(np.int64)
    Sigma_inv = np.asarray(Sigma_inv, dtype=np.float32)
    beta = float(np.asarray(beta).reshape(-1)[0])

    m_pad = t_tiles * P
    xsT = np.ascontiguousarray((x * Sigma_inv).T).astype(NP_FP8)  # [D, N]
    e_sq_full = np.einsum("md,d->m", exemplars * exemplars, Sigma_inv)

    m_loc = M // N_CORES
    in_maps = []
    for c in range(N_CORES):
        e_shard = np.zeros((m_pad, D), dtype=np.float32)
        e_shard[:m_loc] = exemplars[c * m_loc:(c + 1) * m_loc]
        # eTt[p, t*512 + k*128 + m] = e_shard[t*128 + m, k*128 + p]
        eTt = np.ascontiguousarray(
            e_shard.reshape(t_tiles, P, KC, P).transpose(3, 0, 2, 1)
            .reshape(P, t_tiles * D)).astype(NP_FP8)
        lab = labels[c * m_loc:(c + 1) * m_loc]
        onehot = np.zeros((m_pad, CP), dtype=np.float32)
        onehot[np.arange(m_loc), lab] = 1.0
        w_packed = np.ascontiguousarray(
            onehot.reshape(t_tiles, P, CP).transpose(1, 0, 2)
            .reshape(P, t_tiles * CP)).astype(NP_FP8)
        esq = np.zeros(m_pad, dtype=np.float32)
        esq[:m_loc] = e_sq_full[c * m_loc:(c + 1) * m_loc]
        esq_t = esq.reshape(t_tiles, P).T          # [P, t_tiles]
        cb = np.zeros((P, 2 * t_tiles + 2), dtype=np.float32)
        cb[:, 0:t_tiles] = -beta * esq_t
        cb[:, t_tiles:2 * t_tiles] = 56.0 + DELTA - 8.0 * LOG2E * beta * esq_t
        cb[:, 2 * t_tiles] = 2.0 * beta
        cb[:, 2 * t_tiles + 1] = 16.0 * beta * LOG2E
        in_maps.append({
            "eTt": eTt, "xsT": xsT, "w": w_packed, "cb": cb,
        })
    return in_maps


def finalize(core_outs, x, Sigma_inv, beta, gamma):
    """Combine per-core partial logits into the full softmax output."""
    x = np.asarray(x, dtype=np.float32)
    Sigma_inv = np.asarray(Sigma_inv, dtype=np.float32)
    beta = float(np.asarray(beta).reshape(-1)[0])
    gamma = float(np.asarray(gamma).reshape(-1)[0])

    partial = np.zeros_like(core_outs[0], dtype=np.float32)
    for o in core_outs:
        partial += o                                      # [C, N]
    x_sq = np.einsum("nd,d->n", x * x, Sigma_inv)         # [N]
    logits = np.exp(-beta * x_sq)[:, None].astype(np.float32) * partial.T
    z = gamma * logits
    z = z - z.max(axis=1, keepdims=True)
    ez = np.exp(z)
    return (ez / ez.sum(axis=1, keepdims=True)).astype(np.float32)


_NC_CACHE = {}


def kernel(x, exemplars, labels, Sigma_inv, beta, gamma):
    if "nc" not in _NC_CACHE:
        _NC_CACHE["nc"] = build_nc()
    nc = _NC_CACHE["nc"]
    in_maps = make_in_maps(x, exemplars, labels, Sigma_inv, beta, gamma)
    res = bass_utils.run_bass_kernel_spmd(nc, in_maps,
                                          core_ids=list(range(N_CORES)))
    core_outs = [r["out"] for r in res.results]
    return finalize(core_outs, x, Sigma_inv, beta, gamma)


# revision 22
# speedup vs baseline: 1.0556x; 1.0556x over previous
"""Trainium2 Bass kernel for the ExemplarModel (Mahalanobis-kNN attention).

Reference math (N=1024 queries, M=50000 exemplars, D=512, C=10 classes):
    dist[n,m]  = sum_d Sigma_inv[d] * (x[n,d] - e[m,d])^2
    att[n,m]   = exp(-beta * dist[n,m])
    logits[n,c]= segment_sum(att over exemplars with label c)
    out        = softmax(gamma * logits, axis=1)

Distribution: exemplars/labels sharded along M across 8 NeuronCores
(6250 each, zero-padded to 6272 = 49*128); x, Sigma_inv, beta replicated.
Each core computes partial per-class logits
    P[c,n] = sum_m onehot[m,c] * exp(2*beta*cross[n,m] - beta*e_sq[m])
with cross[m,n] = sum_d e[m,d] * (x*Sigma_inv)[n,d].

v2 design (from trace analysis of the 89us v1):
  - e_sq is computed on the HOST (tiny: M*D mults) and shipped as per-tile
    bias columns; the raw f32 exemplar stream, its SWDGE cast-DMA (41us of
    DMA busy) and all DVE square/scale/reduce work (59us busy) are gone.
  - eT is retiled on the host into [128, t*512] so each tile is a
    contiguous 512B-per-partition run; groups of 4 tiles per DMA (2KB
    runs) replace v1's 242B-descriptor loads. First matmul can start at
    ~1us instead of ~13us.
  - exp is split across engines: even tiles on ScalarE (exact Exp -> fp8),
    odd tiles on DVE via a Schraudolph-style trick: bits =
    sat_u8(rne(16*beta*log2e*cross + 56 - 0.46 - 8*log2e*beta*e_sq))
    written as uint8 and bitcast to fp8e4 (HW convert saturates negatives
    to 0 == exp underflow). Each engine handles ~25 tiles (~30us) instead
    of ScalarE doing all 49 (~60us).
  - PE warmup matmuls run during the initial DMA fill to start the DVFS
    ramp (0.65 -> 1.2 -> 2.4 GHz) early.
The PE is the bottleneck: 196 cross + 50 segment DR fp8 matmuls at
~213ns steady (1 col/cycle, the real fp8 peak) ~= 52us.

The host combines: logits[n,c] = exp(-beta*x_sq[n]) * sum_cores P, then
gamma + softmax on the tiny [1024,10] result.
"""

import numpy as np
import ml_dtypes

import concourse.bass as bass
import concourse.bacc as bacc
import concourse.tile as tile
from concourse import mybir
from concourse import bass_utils

# Problem constants (hardcoded per contract; kernel.py must be self-contained).
N = 1024          # queries
M = 50000         # exemplars (global)
D = 512           # feature dim
C = 10            # classes
N_CORES = 8
M_LOC = M // N_CORES          # 6250 exemplars per core
P = 128                       # partitions
T_TILES = (M_LOC + P - 1) // P  # 49 tiles per core
M_PAD = T_TILES * P           # 6272
KC = D // P                   # 4 contraction chunks
CP = 16                       # one-hot pitch (fp8 DoubleRow needs step%16==0)
NH = N // 512                 # 2 matmul free-dim halves
G_TILES = 4                   # eT tiles per DMA group
N_GROUPS = (T_TILES + G_TILES - 1) // G_TILES
N_WARM = 48                   # PE warmup matmuls during DMA fill

LOG2E = float(np.log2(np.e))
DELTA = -0.46                 # Schraudolph magic offset for e4m3 (tuned)

FP32 = mybir.dt.float32
FP8 = mybir.dt.float8e4
U8 = mybir.dt.uint8
NP_FP8 = ml_dtypes.float8_e4m3


def build_nc(t_tiles=T_TILES, n=N, debug=False):
    """Build the per-core Bass program (SPMD: same program, per-core data)."""
    nc = bacc.Bacc("TRN2", target_bir_lowering=False, debug=debug,
                   num_devices=N_CORES)
    nh = n // 512

    eTt_dram = nc.dram_tensor("eTt", [P, t_tiles * D], FP8, kind="ExternalInput")
    xsT_dram = nc.dram_tensor("xsT", [D, n], FP8, kind="ExternalInput")
    w_dram = nc.dram_tensor("w", [P, t_tiles * CP], FP8, kind="ExternalInput")
    # cb = [ba | bd | sc] packed: one DMA covers every f32 constant
    cb_dram = nc.dram_tensor("cb", [P, 2 * t_tiles + 2], FP32,
                             kind="ExternalInput")
    out_dram = nc.dram_tensor("out", [C, n], FP32, kind="ExternalOutput")

    with tile.TileContext(nc) as tc:
        with (
            tc.tile_pool(name="const", bufs=1) as const_pool,
            tc.tile_pool(name="att", bufs=4) as att_pool,
            tc.tile_pool(name="crossp", bufs=3, space="PSUM") as cross_pool,
            tc.tile_pool(name="logitp", bufs=1, space="PSUM") as logit_pool,
        ):
            # ---- one-time preamble ----
            # xsT first on the Scalar HWDGE queue (feeds the first matmul);
            # one tile per 256-d pair chunk so the first matmul only waits
            # on chunks 0-1.
            # Scalar-queue order is latency-driven: the first cross matmul
            # needs xsT chunk-pair 0, then pair 1, then the act constants,
            # then w (first needed by the t==2 segment matmul). One DMA
            # each to minimize issue overhead and semaphore chaining.
            xsT_p0 = const_pool.tile([P, 2 * n], FP8, tag="xsTp0")
            xsT_p1 = const_pool.tile([P, 2 * n], FP8, tag="xsTp1")
            xsT_tiles = [xsT_p0, xsT_p1]
            xsT_view = xsT_dram.ap().rearrange("(k p) n -> p k n", p=P)
            ba = const_pool.tile([P, t_tiles], FP32, tag="ba")
            bd = const_pool.tile([P, t_tiles], FP32, tag="bd")
            sc = const_pool.tile([P, 2], FP32, tag="sc")
            w_f8 = const_pool.tile([P, t_tiles * CP], FP8, tag="w8")
            nc.scalar.dma_start(sc[:], cb_dram[:, 2 * t_tiles:2 * t_tiles + 2])
            nc.scalar.dma_start(
                xsT_p0[:].rearrange("p (k n) -> p k n", n=n),
                xsT_view[:, 0:2, :])
            nc.scalar.dma_start(
                xsT_p1[:].rearrange("p (k n) -> p k n", n=n),
                xsT_view[:, 2:4, :])
            nc.scalar.dma_start(ba[:], cb_dram[:, 0:t_tiles])
            nc.scalar.dma_start(bd[:], cb_dram[:, t_tiles:2 * t_tiles])
            nc.scalar.dma_start(w_f8[:], w_dram[:])
            xsT_pair_aps = [t_[:].rearrange("p (k n) -> p k n", n=n)
                            for t_ in xsT_tiles]

            # Tiled exemplar loads on the Sync HWDGE queue: graded group
            # sizes — small first groups so the early tiles land with low
            # latency, big groups later for issue/semaphore efficiency.
            group_sizes = [1, 2, 4, 8]
            while sum(group_sizes) + 8 <= t_tiles:
                group_sizes.append(8)
            rem = t_tiles - sum(group_sizes)
            if rem:
                group_sizes.append(rem)
            eT_groups = []
            tile2group = []
            off = 0
            for g, gt in enumerate(group_sizes):
                tile_g = const_pool.tile([P, gt * D], FP8, tag=f"eT{g}")
                nc.sync.dma_start(
                    tile_g[:], eTt_dram[:, off * D:(off + gt) * D])
                for lo in range(gt):
                    tile2group.append((g, lo))
                eT_groups.append(tile_g)
                off += gt

            # Full-width PSUM tile: [:CP] is the logits accumulator; the
            # warmup matmuls scribble on it first (the t==2 start=True
            # segment matmul resets its region afterwards).
            logits_full = logit_pool.tile([P, n], FP32)
            logits_ps = logits_full[:CP, :]

            # PE warmup: narrow DR matmuls on a zeroed scratch tile to start
            # the clock ramp while the first DMAs land. Sized to bridge the
            # PE from the post-preamble start (~6.9us) to first-data-ready
            # (~9.5us) without a gap (the DVFS clock decays within ~1us of
            # idle), while cheap enough not to delay the first real matmul.
            scratch = const_pool.tile([P, 2 * P], FP8, tag="scr")
            nc.gpsimd.memset(scratch[:], 0)
            scr_pairs = scratch[:].rearrange("p (i n) -> p i n", i=2)
            for _ in range(N_WARM):
                nc.tensor.matmul(
                    logits_full[:32, :64], lhsT=scr_pairs[:, :, :32],
                    rhs=scr_pairs[:, :, :64], start=True, stop=True,
                    perf_mode=mybir.MatmulPerfMode.DoubleRow,
                    skip_group_check=True)

            w_pairs = w_f8[:].rearrange("p (t c) -> p t c", c=CP)

            # ---- main loop over exemplar tiles ----
            # att for two consecutive tiles shares one buffer so the segment
            # matmul can consume both via one fp8 DoubleRow op; it is issued
            # two tiles behind so the PE never waits on the act engines.
            att_pairs = []
            att_cur = None
            for t in range(t_tiles):
                g, lo = tile2group[t]
                eT_t = eT_groups[g][:, lo * D:(lo + 1) * D].rearrange(
                    "p (k m) -> p k m", m=P)

                # cross[m, n] = sum_d e[m,d] * xs[n,d]
                # fp8 DoubleRow: each matmul consumes a pair of 128-d chunks
                cross_ps = cross_pool.tile([P, n], FP32, tag="cross")
                for j in range(KC // 2):
                    for h in range(nh):
                        nc.tensor.matmul(
                            cross_ps[:, h * 512:(h + 1) * 512],
                            lhsT=eT_t[:, 2 * j:2 * j + 2, :],
                            rhs=xsT_pair_aps[j][:, :, h * 512:(h + 1) * 512],
                            start=(j == 0), stop=(j == KC // 2 - 1),
                            perf_mode=mybir.MatmulPerfMode.DoubleRow)

                # logits[c, n] += onehot[m, c]^T @ att[m, n]  (pair t//2 - 1)
                if t % 2 == 0 and len(att_pairs) >= 1 and not att_pairs[-1][1]:
                    p_idx, _ = att_pairs[-1]
                    att_pairs[-1] = (p_idx, True)
                    pr = p_idx[:].rearrange("p (i n) -> p i n", i=2)
                    for h in range(nh):
                        nc.tensor.matmul(
                            logits_ps[:, h * 512:(h + 1) * 512],
                            lhsT=w_pairs[:, t - 2:t, :],
                            rhs=pr[:, :, h * 512:(h + 1) * 512],
                            start=(t == 2), stop=False,
                            perf_mode=mybir.MatmulPerfMode.DoubleRow,
                            skip_group_check=True)

                # att = exp(2*beta*cross - beta*e_sq), alternating engines:
                # even tiles exact Exp on ScalarE (fp8 out), odd tiles
                # Schraudolph bits on DVE (uint8 out, bitcast fp8).
                if t % 2 == 0:
                    att_cur = att_pool.tile([P, 2 * n], FP8, tag="att")
                    att_pairs.append((att_cur, False))
                half = att_cur[:, (t % 2) * n:(t % 2 + 1) * n]
                if t % 2 == 0:
                    nc.scalar.activation(half, cross_ps[:],
                                         mybir.ActivationFunctionType.Exp,
                                         bias=ba[:, t:t + 1],
                                         scale=sc[:, 0:1])
                else:
                    nc.vector.tensor_scalar(
                        half.bitcast(U8), cross_ps[:],
                        sc[:, 1:2], bd[:, t:t + 1],
                        mybir.AluOpType.mult, mybir.AluOpType.add)

            # drain remaining segment matmuls
            n_pairs = t_tiles // 2
            last_single = (t_tiles % 2 == 1)
            for pi in range(len(att_pairs)):
                p_idx, done = att_pairs[pi]
                if done:
                    continue
                if pi < n_pairs:
                    pr = p_idx[:].rearrange("p (i n) -> p i n", i=2)
                    for h in range(nh):
                        nc.tensor.matmul(
                            logits_ps[:, h * 512:(h + 1) * 512],
                            lhsT=w_pairs[:, 2 * pi:2 * pi + 2, :],
                            rhs=pr[:, :, h * 512:(h + 1) * 512],
                            start=(pi == 0),
                            stop=(not last_single and pi == len(att_pairs) - 1),
                            perf_mode=mybir.MatmulPerfMode.DoubleRow,
                            skip_group_check=True)
                else:  # leftover single tile (first half of the pair buffer)
                    out_sb = const_pool.tile([C, n], FP32, tag="out")
                    for h in range(nh):
                        nc.tensor.matmul(
                            logits_ps[:, h * 512:(h + 1) * 512],
                            lhsT=w_f8[:, (t_tiles - 1) * CP:t_tiles * CP],
                            rhs=p_idx[:, h * 512:(h + 1) * 512],
                            start=False, stop=(h == nh - 1),
                            skip_group_check=True)
                        # epilogue for this n-half overlaps the next half's
                        # segment matmul
                        nc.vector.tensor_copy(
                            out_sb[:, h * 512:(h + 1) * 512],
                            logits_ps[:C, h * 512:(h + 1) * 512])
                        nc.sync.dma_start(
                            out_dram[:, h * 512:(h + 1) * 512],
                            out_sb[:, h * 512:(h + 1) * 512])

    nc.compile()
    return nc


def make_in_maps(x, exemplars, labels, Sigma_inv, beta, gamma,
                 t_tiles=T_TILES):
    """Shard the full inputs into per-core in_maps (host-side glue)."""
    x = np.asarray(x, dtype=np.float32)
    exemplars = np.asarray(exemplars, dtype=np.float32)
    labels = np.asarray(labels).astype(np.int64)
    Sigma_inv = np.asarray(Sigma_inv, dtype=np.float32)
    beta = float(np.asarray(beta).reshape(-1)[0])

    m_pad = t_tiles * P
    xsT = np.ascontiguousarray((x * Sigma_inv).T).astype(NP_FP8)  # [D, N]
    e_sq_full = np.einsum("md,d->m", exemplars * exemplars, Sigma_inv)

    m_loc = M // N_CORES
    in_maps = []
    for c in range(N_CORES):
        e_shard = np.zeros((m_pad, D), dtype=np.float32)
        e_shard[:m_loc] = exemplars[c * m_loc:(c + 1) * m_loc]
        # eTt[p, t*512 + k*128 + m] = e_shard[t*128 + m, k*128 + p]
        eTt = np.ascontiguousarray(
            e_shard.reshape(t_tiles, P, KC, P).transpose(3, 0, 2, 1)
            .reshape(P, t_tiles * D)).astype(NP_FP8)
        lab = labels[c * m_loc:(c + 1) * m_loc]
        onehot = np.zeros((m_pad, CP), dtype=np.float32)
        onehot[np.arange(m_loc), lab] = 1.0
        w_packed = np.ascontiguousarray(
            onehot.reshape(t_tiles, P, CP).transpose(1, 0, 2)
            .reshape(P, t_tiles * CP)).astype(NP_FP8)
        esq = np.zeros(m_pad, dtype=np.float32)
        esq[:m_loc] = e_sq_full[c * m_loc:(c + 1) * m_loc]
        esq_t = esq.reshape(t_tiles, P).T          # [P, t_tiles]
        cb = np.zeros((P, 2 * t_tiles + 2), dtype=np.float32)
        cb[:, 0:t_tiles] = -beta * esq_t
        cb[:, t_tiles:2 * t_tiles] = 56.0 + DELTA - 8.0 * LOG2E * beta * esq_t
        cb[:, 2 * t_tiles] = 2.0 * beta
        cb[:, 2 * t_tiles + 1] = 16.0 * beta * LOG2E
        in_maps.append({
            "eTt": eTt, "xsT": xsT, "w": w_packed, "cb": cb,
        })
    return in_maps


def finalize(core_outs, x, Sigma_inv, beta, gamma):
    """Combine per-core partial logits into the full softmax output."""
    x = np.asarray(x, dtype=np.float32)
    Sigma_inv = np.asarray(Sigma_inv, dtype=np.float32)
    beta = float(np.asarray(beta).reshape(-1)[0])
    gamma = float(np.asarray(gamma).reshape(-1)[0])

    partial = np.zeros_like(core_outs[0], dtype=np.float32)
    for o in core_outs:
        partial += o                                      # [C, N]
    x_sq = np.einsum("nd,d->n", x * x, Sigma_inv)         # [N]
    logits = np.exp(-beta * x_sq)[:, None].astype(np.float32) * partial.T
    z = gamma * logits
    z = z - z.max(axis=1, keepdims=True)
    ez = np.exp(z)
    return (ez / ez.sum(axis=1, keepdims=True)).astype(np.float32)


_NC_CACHE = {}


def kernel(x, exemplars, labels, Sigma_inv, beta, gamma):
    if "nc" not in _NC_CACHE:
        _NC_CACHE["nc"] = build_nc()
    nc = _NC_CACHE["nc"]
    in_maps = make_in_maps(x, exemplars, labels, Sigma_inv, beta, gamma)
    res = bass_utils.run_bass_kernel_spmd(nc, in_maps,
                                          core_ids=list(range(N_CORES)))
    core_outs = [r["out"] for r in res.results]
    return finalize(core_outs, x, Sigma_inv, beta, gamma)
